# revision 21
# baseline (speedup 1.0000x reference)
"""EyesMouthLoss Trainium2 kernel.

loss = mean(|pred-target| * (1 + 299*clip(eye_mask+mouth_mask, 0, 1)))

Sharding: pure data-parallel over B=16 -> 2 batches per core on 8 cores.
Host sums the 8 per-core partial scalars (the final all-reduce).

The masks depend only on `landmarks` (tiny: 16x68x2 ints), so the host
precomputes the priority field, quantizes it to u8 (256 KB/core next to
the 12.6 MB/core of fp32 pred/target), and ACT dequantizes it to the
bf16 weight w = 1 + (299/255)*w' once per chunk.

pred/target are shipped interleaved in a host-transposed layout
[k, {pred,targ}, row, c*x] so each 128-row chunk is ONE casting SWDGE
DMA (fp32 HBM -> bf16 SBUF, 6 KB -> 3 KB contiguous descriptors) whose
completion semaphore is exactly what the chunk's compute waits on.
Per chunk the compute is a 3-op bf16 stream:

    d   = pred - target     DVE tensor_tensor (bf16, full rate)
    a   = |d|               ACT Abs
    g   = a * w             DVE scalar_tensor_tensor, w broadcast over
                            channels, fp32 accum_out = weighted row-sum

The STT is emitted one unit behind its SUB so the DVE queue head never
waits on the cross-engine ABS; the last two units are split into
x-halves (loads and compute) so the tail chain after the final DMA
completion is short.  The [128, 10] fp32 accumulator tile is the only
output; the host applies the final 1/N over the 8 cores.
"""

import sys

sys.path.insert(0, "/opt/trn_rl_repo")

from contextlib import ExitStack

import numpy as np

import concourse.bass as bass
import concourse.tile as tile
from concourse import bacc, mybir
from concourse.bass_utils import run_bass_kernel_spmd

B, C, H, W = 16, 3, 512, 512
NCORES = 8
BPC = B // NCORES  # batches per core
NCHUNK = 4  # 512 rows = 4 x 128 partitions
CW = C * W
NSPLIT = 8  # trailing units computed in x-halves for a short tail
RADIUS = 15.0
HALF = 14  # region strictly zero for |dx| >= 15
EYE = (36, 48)
MOUTH = (48, 68)
WEIGHT = 300.0
NTOT = float(B * C * H * W)
FP32 = mybir.dt.float32
BF16 = mybir.dt.bfloat16
U8 = mybir.dt.uint8
Alu = mybir.AluOpType
Act = mybir.ActivationFunctionType

_STENCIL = None


def _stencil():
    global _STENCIL
    if _STENCIL is None:
        d = np.arange(2 * HALF + 1, dtype=np.float32) - HALF
        r = np.sqrt(d[:, None] ** 2 + d[None, :] ** 2)
        _STENCIL = np.clip(1.0 - r / RADIUS, 0.0, 1.0).astype(np.float32)
    return _STENCIL


def _priority_u8(landmarks):
    """w'[b,y,x] = round(255*clip(eye+mouth, 0, 1)), computed on host."""
    st = _stencil()
    w = np.empty((B, H, W), np.uint8)
    for b in range(B):
        fields = np.zeros((2, H, W), np.float32)
        for field, (lo, hi) in zip(fields, (EYE, MOUTH)):
            for cx, cy in landmarks[b, lo:hi]:
                cx = int(min(max(int(cx), 0), W - 1))
                cy = int(min(max(int(cy), 0), H - 1))
                y0, y1 = max(0, cy - HALF), min(H - 1, cy + HALF)
                x0, x1 = max(0, cx - HALF), min(W - 1, cx + HALF)
                sy0, sx0 = y0 - (cy - HALF), x0 - (cx - HALF)
                np.maximum(
                    field[y0 : y1 + 1, x0 : x1 + 1],
                    st[sy0 : sy0 + y1 - y0 + 1, sx0 : sx0 + x1 - x0 + 1],
                    out=field[y0 : y1 + 1, x0 : x1 + 1],
                )
        w[b] = np.rint(
            255.0 * np.minimum(fields[0] + fields[1], 1.0)
        ).astype(np.uint8)
    return w


def _build():
    """Build the SPMD Bass program (shared by all cores; data-parallel)."""
    nc = bacc.Bacc(None)
    # host layout: [bi, k, row(128), {pred,targ}, c*x] — each row carries its
    # pred plane then its targ plane contiguously (one 12 KB descriptor)
    pt_p = nc.declare_dram_parameter(
        "pt", [BPC, NCHUNK, 128, 2, CW], FP32, isOutput=False
    )
    wgt_p = nc.declare_dram_parameter("wgt", [BPC, NCHUNK, 128, W], U8, isOutput=False)
    nu = BPC * NCHUNK
    nfull = nu - NSPLIT
    nacc = nfull + 2 * NSPLIT
    out_p = nc.declare_dram_parameter("out", [128, nacc], FP32, isOutput=True)

    with tile.TileContext(nc) as tc, ExitStack() as ctx:
        stat_pool = ctx.enter_context(tc.tile_pool(name="stat", bufs=2))
        load_pool = ctx.enter_context(tc.tile_pool(name="load", bufs=2))

        units = [(bi, k) for bi in range(BPC) for k in range(NCHUNK)]
        rs = stat_pool.tile([128, nacc], FP32)

        w_u8 = load_pool.tile([128, BPC, NCHUNK, W], U8, tag="w_u8")
        w_e = load_pool.tile([128, BPC, NCHUNK, W], BF16, tag="w_e")
        pt_ts = [
            load_pool.tile([128, NCHUNK, 2, CW], BF16, tag="pt", name=f"pt{bi}")
            for bi in range(BPC)
        ]

        def xsl(xh):
            if xh is None:
                return slice(None)
            return slice(xh * (W // 2), (xh + 1) * (W // 2))

        def sb(u, t, xh=None):  # SBUF view [128, C, Wslice] of pred/targ plane
            bi, k = units[u]
            v = pt_ts[bi][:, k, t, :].rearrange("p (c x) -> p c x", c=C)
            return v[:, :, xsl(xh)]

        def load(u, xh=None):
            bi, k = units[u]
            if xh is None:
                nc.gpsimd.dma_start(pt_ts[bi][:, k, :, :], pt_p[bi, k])
            else:
                for t in (0, 1):
                    out_v = pt_ts[bi][:, k, t, :].rearrange(
                        "p (c x) -> p c x", c=C
                    )[:, :, xsl(xh)]
                    in_v = pt_p[bi, k, :, t].rearrange(
                        "p (c x) -> p c x", c=C
                    )[:, :, xsl(xh)]
                    nc.gpsimd.dma_start(out_v, in_v)

        def wexp(u):
            bi, k = units[u]
            nc.scalar.activation(
                w_e[:, bi, k, :], w_u8[:, bi, k, :], Act.Identity,
                bias=1.0, scale=(WEIGHT - 1.0) / 255.0,
            )

        def sub(u, xh=None):
            nc.vector.tensor_tensor(
                sb(u, 0, xh), sb(u, 0, xh), sb(u, 1, xh), op=Alu.subtract
            )

        def abs_(u, xh=None):
            nc.scalar.activation(sb(u, 1, xh), sb(u, 0, xh), Act.Abs)

        def stt(u, xh=None, col=0):
            bi, k = units[u]
            wn = W if xh is None else W // 2
            wb = (
                w_e[:, bi, k, xsl(xh)]
                .broadcast_to([128, wn, C])
                .rearrange("p x c -> p c x")
            )
            nc.vector.scalar_tensor_tensor(
                sb(u, 0, xh), sb(u, 1, xh), 1.0, wb,
                op0=Alu.mult, op1=Alu.mult,
                accum_out=rs[:, col : col + 1],
            )

        # ---- emission: loads first (SWDGE FIFO = arrival order), w' on the
        # idle SP HWDGE ring, then the software-pipelined compute stream ----
        load(0)
        for bi in range(BPC):
            nc.sync.dma_start(
                w_u8[:, bi, :, :], wgt_p[bi].rearrange("k p x -> p k x")
            )
        for u in range(1, nu):
            if u < nfull:
                load(u)
            else:
                load(u, xh=0)
                load(u, xh=1)

        if nfull > 0:
            wexp(0)
            sub(0)
            abs_(0)
            for u in range(1, nfull):
                wexp(u)
                sub(u)
                abs_(u)
                stt(u - 1, col=u - 1)
            stt(nfull - 1, col=nfull - 1)
        for i, u in enumerate(range(nfull, nu)):
            wexp(u)
        for i, u in enumerate(range(nfull, nu)):
            for xh in (0, 1):
                sub(u, xh=xh)
                abs_(u, xh=xh)
            for xh in (0, 1):
                stt(u, xh=xh, col=nfull + 2 * i + xh)

        nc.sync.dma_start(out_p[:, :], rs[:])

    return nc


def _pack_pt(pred, targ):
    """-> [B, NCHUNK, 128, 2, CW]: per row, pred plane then targ plane."""
    def t(a):
        return a.reshape(B, C, NCHUNK, 128, W).transpose(0, 2, 3, 1, 4)

    pt = np.stack([t(pred), t(targ)], axis=3)  # [B, NCHUNK, 128, 2, C, W]
    return np.ascontiguousarray(pt).reshape(B, NCHUNK, 128, 2, CW)


def run(inputs, trace=False):
    pred = np.ascontiguousarray(inputs["pred"], dtype=np.float32)
    targ = np.ascontiguousarray(inputs["target"], dtype=np.float32)
    lms = np.asarray(inputs["landmarks"])
    assert pred.shape == (B, C, H, W) and targ.shape == (B, C, H, W)

    w = _priority_u8(lms).reshape(B, NCHUNK, 128, W)
    pt = _pack_pt(pred, targ)

    nc = _build()
    nc.finalize()
    in_maps = [
        {
            "pt": pt[i * BPC : (i + 1) * BPC],
            "wgt": w[i * BPC : (i + 1) * BPC],
        }
        for i in range(NCORES)
    ]
    res = run_bass_kernel_spmd(nc, in_maps, list(range(NCORES)), trace=trace)
    total = 0.0
    for i in range(NCORES):
        total += res.results[i]["out"].astype(np.float64).sum()
    return np.float32(total / NTOT), res


def kernel(pred, target, landmarks):
    out, _ = run({"pred": pred, "target": target, "landmarks": landmarks})
    return out


# revision 22
# speedup vs baseline: 1.0211x; 1.0211x over previous
"""EyesMouthLoss Trainium2 kernel.

loss = mean(|pred-target| * (1 + 299*clip(eye_mask+mouth_mask, 0, 1)))

Sharding: pure data-parallel over B=16 -> 2 batches per core on 8 cores.
Host sums the 8 per-core partial scalars (the final all-reduce).

The masks depend only on `landmarks` (tiny: 16x68x2 ints), so the host
precomputes the priority field, quantizes it to u8 (256 KB/core next to
the 12.6 MB/core of fp32 pred/target), and ACT dequantizes it to the
bf16 weight w = 1 + (299/255)*w' once per chunk.

pred/target are shipped interleaved in a host-transposed layout
[k, {pred,targ}, row, c*x] so each 128-row chunk is ONE casting SWDGE
DMA (fp32 HBM -> bf16 SBUF, 6 KB -> 3 KB contiguous descriptors) whose
completion semaphore is exactly what the chunk's compute waits on.
Per chunk the compute is a 3-op bf16 stream:

    d   = pred - target     DVE tensor_tensor (bf16, full rate)
    a   = |d|               ACT Abs
    g   = a * w             DVE scalar_tensor_tensor, w broadcast over
                            channels, fp32 accum_out = weighted row-sum

The STT is emitted one unit behind its SUB so the DVE queue head never
waits on the cross-engine ABS; the last two units are split into
x-halves (loads and compute) so the tail chain after the final DMA
completion is short.  The [128, 10] fp32 accumulator tile is the only
output; the host applies the final 1/N over the 8 cores.
"""

import sys

sys.path.insert(0, "/opt/trn_rl_repo")

from contextlib import ExitStack

import numpy as np

import concourse.bass as bass
import concourse.tile as tile
from concourse import bacc, mybir
from concourse.bass_utils import run_bass_kernel_spmd

B, C, H, W = 16, 3, 512, 512
NCORES = 8
BPC = B // NCORES  # batches per core
NCHUNK = 4  # 512 rows = 4 x 128 partitions
CW = C * W
NSPLIT = 4  # trailing units computed in x-halves for a short tail
RADIUS = 15.0
HALF = 14  # region strictly zero for |dx| >= 15
EYE = (36, 48)
MOUTH = (48, 68)
WEIGHT = 300.0
NTOT = float(B * C * H * W)
FP32 = mybir.dt.float32
BF16 = mybir.dt.bfloat16
U8 = mybir.dt.uint8
Alu = mybir.AluOpType
Act = mybir.ActivationFunctionType

_STENCIL = None


def _stencil():
    global _STENCIL
    if _STENCIL is None:
        d = np.arange(2 * HALF + 1, dtype=np.float32) - HALF
        r = np.sqrt(d[:, None] ** 2 + d[None, :] ** 2)
        _STENCIL = np.clip(1.0 - r / RADIUS, 0.0, 1.0).astype(np.float32)
    return _STENCIL


def _priority_u8(landmarks):
    """w'[b,y,x] = round(255*clip(eye+mouth, 0, 1)), computed on host."""
    st = _stencil()
    w = np.empty((B, H, W), np.uint8)
    for b in range(B):
        fields = np.zeros((2, H, W), np.float32)
        for field, (lo, hi) in zip(fields, (EYE, MOUTH)):
            for cx, cy in landmarks[b, lo:hi]:
                cx = int(min(max(int(cx), 0), W - 1))
                cy = int(min(max(int(cy), 0), H - 1))
                y0, y1 = max(0, cy - HALF), min(H - 1, cy + HALF)
                x0, x1 = max(0, cx - HALF), min(W - 1, cx + HALF)
                sy0, sx0 = y0 - (cy - HALF), x0 - (cx - HALF)
                np.maximum(
                    field[y0 : y1 + 1, x0 : x1 + 1],
                    st[sy0 : sy0 + y1 - y0 + 1, sx0 : sx0 + x1 - x0 + 1],
                    out=field[y0 : y1 + 1, x0 : x1 + 1],
                )
        w[b] = np.rint(
            255.0 * np.minimum(fields[0] + fields[1], 1.0)
        ).astype(np.uint8)
    return w


def _build():
    """Build the SPMD Bass program (shared by all cores; data-parallel)."""
    nc = bacc.Bacc(None)
    # host layout: [bi, k, row(128), {pred,targ}, c*x] — each row carries its
    # pred plane then its targ plane contiguously (one 12 KB descriptor)
    pt_p = nc.declare_dram_parameter(
        "pt", [BPC, NCHUNK, 128, 2, CW], FP32, isOutput=False
    )
    wgt_p = nc.declare_dram_parameter("wgt", [BPC, NCHUNK, 128, W], U8, isOutput=False)
    nu = BPC * NCHUNK
    nfull = nu - NSPLIT
    nacc = nfull + 2 * NSPLIT
    out_p = nc.declare_dram_parameter("out", [128, nacc], FP32, isOutput=True)

    with tile.TileContext(nc) as tc, ExitStack() as ctx:
        stat_pool = ctx.enter_context(tc.tile_pool(name="stat", bufs=2))
        load_pool = ctx.enter_context(tc.tile_pool(name="load", bufs=2))

        units = [(bi, k) for bi in range(BPC) for k in range(NCHUNK)]
        rs = stat_pool.tile([128, nacc], FP32)

        w_u8 = load_pool.tile([128, BPC, NCHUNK, W], U8, tag="w_u8")
        w_e = load_pool.tile([128, BPC, NCHUNK, W], BF16, tag="w_e")
        pt_ts = [
            load_pool.tile([128, NCHUNK, 2, CW], BF16, tag="pt", name=f"pt{bi}")
            for bi in range(BPC)
        ]

        def xsl(xh):
            if xh is None:
                return slice(None)
            return slice(xh * (W // 2), (xh + 1) * (W // 2))

        def sb(u, t, xh=None):  # SBUF view [128, C, Wslice] of pred/targ plane
            bi, k = units[u]
            v = pt_ts[bi][:, k, t, :].rearrange("p (c x) -> p c x", c=C)
            return v[:, :, xsl(xh)]

        def load(u, xh=None):
            bi, k = units[u]
            if xh is None:
                nc.gpsimd.dma_start(pt_ts[bi][:, k, :, :], pt_p[bi, k])
            else:
                for t in (0, 1):
                    out_v = pt_ts[bi][:, k, t, :].rearrange(
                        "p (c x) -> p c x", c=C
                    )[:, :, xsl(xh)]
                    in_v = pt_p[bi, k, :, t].rearrange(
                        "p (c x) -> p c x", c=C
                    )[:, :, xsl(xh)]
                    nc.gpsimd.dma_start(out_v, in_v)

        def wexp(u):
            bi, k = units[u]
            nc.scalar.activation(
                w_e[:, bi, k, :], w_u8[:, bi, k, :], Act.Identity,
                bias=1.0, scale=(WEIGHT - 1.0) / 255.0,
            )

        def sub(u, xh=None):
            nc.vector.tensor_tensor(
                sb(u, 0, xh), sb(u, 0, xh), sb(u, 1, xh), op=Alu.subtract
            )

        def abs_(u, xh=None):
            nc.scalar.activation(sb(u, 1, xh), sb(u, 0, xh), Act.Abs)

        def stt(u, xh=None, col=0):
            bi, k = units[u]
            wn = W if xh is None else W // 2
            wb = (
                w_e[:, bi, k, xsl(xh)]
                .broadcast_to([128, wn, C])
                .rearrange("p x c -> p c x")
            )
            nc.vector.scalar_tensor_tensor(
                sb(u, 0, xh), sb(u, 1, xh), 1.0, wb,
                op0=Alu.mult, op1=Alu.mult,
                accum_out=rs[:, col : col + 1],
            )

        # ---- emission: loads first (SWDGE FIFO = arrival order), w' on the
        # idle SP HWDGE ring, then the software-pipelined compute stream ----
        load(0)
        for bi in range(BPC):
            nc.sync.dma_start(
                w_u8[:, bi, :, :], wgt_p[bi].rearrange("k p x -> p k x")
            )
        for u in range(1, nu):
            if u < nfull:
                load(u)
            else:
                load(u, xh=0)
                load(u, xh=1)

        if nfull > 0:
            wexp(0)
            sub(0)
            abs_(0)
            for u in range(1, nfull):
                wexp(u)
                sub(u)
                abs_(u)
                stt(u - 1, col=u - 1)
            stt(nfull - 1, col=nfull - 1)
        for i, u in enumerate(range(nfull, nu)):
            wexp(u)
        for i, u in enumerate(range(nfull, nu)):
            for xh in (0, 1):
                sub(u, xh=xh)
                abs_(u, xh=xh)
            for xh in (0, 1):
                stt(u, xh=xh, col=nfull + 2 * i + xh)

        nc.sync.dma_start(out_p[:, :], rs[:])

    return nc


def _pack_pt(pred, targ):
    """-> [B, NCHUNK, 128, 2, CW]: per row, pred plane then targ plane."""
    def t(a):
        return a.reshape(B, C, NCHUNK, 128, W).transpose(0, 2, 3, 1, 4)

    pt = np.stack([t(pred), t(targ)], axis=3)  # [B, NCHUNK, 128, 2, C, W]
    return np.ascontiguousarray(pt).reshape(B, NCHUNK, 128, 2, CW)


def run(inputs, trace=False):
    pred = np.ascontiguousarray(inputs["pred"], dtype=np.float32)
    targ = np.ascontiguousarray(inputs["target"], dtype=np.float32)
    lms = np.asarray(inputs["landmarks"])
    assert pred.shape == (B, C, H, W) and targ.shape == (B, C, H, W)

    w = _priority_u8(lms).reshape(B, NCHUNK, 128, W)
    pt = _pack_pt(pred, targ)

    nc = _build()
    nc.finalize()
    in_maps = [
        {
            "pt": pt[i * BPC : (i + 1) * BPC],
            "wgt": w[i * BPC : (i + 1) * BPC],
        }
        for i in range(NCORES)
    ]
    res = run_bass_kernel_spmd(nc, in_maps, list(range(NCORES)), trace=trace)
    total = 0.0
    for i in range(NCORES):
        total += res.results[i]["out"].astype(np.float64).sum()
    return np.float32(total / NTOT), res


def kernel(pred, target, landmarks):
    out, _ = run({"pred": pred, "target": target, "landmarks": landmarks})
    return out
